# revision 5
# baseline (speedup 1.0000x reference)
"""DigitCaps dynamic-routing kernel for 8 TRN2 NeuronCores (v2).

Math refactor (u_hat is NEVER materialized - it would be 189 MB):
  u_hat[b,r,c,d] = sum_i W[r,c,d,i] * u[b,r,i]
  softmax over r without max-subtraction (b_ij values are O(1)):
      c_ij[r,c,d] = exp(b[r,c,d]) / Z[c,d],  Z = sum_r exp(b)
  s[b,c,d]  = (sum_{r,i} (exp(b) * W)[r,c,d,i] u[b,r,i]) / Z[c,d]
  v = squash(s) = s|s|/(1+s^2)  (eps dropped; rel err ~1e-4)
    = y|y| / (Z^2 + y^2) with y the UN-normalized s  (Z-division folded in)
  b += (1/B) sum_b t[b,r,c] v[b,c,d],  t[b,r,c] = sum_i (sum_d W)[r,c,i] u[b,r,i]
       (t is iteration-invariant -> computed once, hidden under AllReduce 0)

Sharding: routes (R=1152) split across 8 cores (144 each). Per iteration one
fused bf16 AllReduce carries the partial s' (B x C*D) and partial Z.

v2 changes vs v1:
  - bf16 on the wire (uT, Wtb, sel) + bf16 AllReduce payloads
  - b_ij layout [(k,rp)=128, (c,d)] + [16-route tail] so each b-update
    (c,bh) pair is ONE M=128 matmul: 40 matmuls/update instead of 60
  - t stored [b, (c,k,rp)] so b-update lhsT slices are contiguous
  - squash via abs_max + reciprocal_approx_fast (no Sqrt -> no ACT table
    swaps; approx recip is ~5x faster than nc.vector.reciprocal)
  - dummy 4-byte AllReduce first thing to absorb the rank-entry barrier
  - program order puts iter-0 s-matmuls before all setup so AR0 starts ASAP
"""

import os
import numpy as np

B, R, C, D, I = 256, 1152, 10, 16, 8
CD = C * D                 # 160
NCORES = 8
RL = R // NCORES           # 144 routes per core
NCHUNK = RL * I // 128     # 9 K-chunks of 128
KMAIN = 8                  # chunks whose routes live in b_main (8*16=128)
NITER = 3

_CACHE = {}


def _build_program():
    from contextlib import ExitStack

    import concourse.bass as bass
    import concourse.bacc as bacc
    import concourse.mybir as mybir
    import concourse.tile as tile

    f32 = mybir.dt.float32
    bf16 = mybir.dt.bfloat16
    AF = mybir.ActivationFunctionType
    ALU = mybir.AluOpType

    nc = bacc.Bacc(None, num_devices=NCORES)

    # bf16 blob: uTb | Wtb | sel   (cols 2304 | 1440 | 1152)
    o_uT, o_Wtb, o_sel = 0, NCHUNK * B, NCHUNK * B + NCHUNK * CD
    DWB = o_sel + NCHUNK * 128
    # f32 blob: Wt | mask2        (cols 1440 | 16)
    o_Wt, o_mk = 0, NCHUNK * CD
    DWF = o_mk + 16
    data_b = nc.declare_dram_parameter("data_b", [128, DWB], bf16, isOutput=False)
    data_f = nc.declare_dram_parameter("data_f", [128, DWF], f32, isOutput=False)
    out_d = nc.declare_dram_parameter("out", [B, CD], f32, isOutput=True)

    rgroups = [list(range(NCORES))]

    with tile.TileContext(nc) as tc, ExitStack() as ctx:
        singles = ctx.enter_context(tc.tile_pool(name="singles", bufs=1))
        wcpool = ctx.enter_context(tc.tile_pool(name="wc", bufs=3))
        stpool = ctx.enter_context(tc.tile_pool(name="stage", bufs=2))
        work = ctx.enter_context(tc.tile_pool(name="work", bufs=2))
        ps_s = ctx.enter_context(tc.tile_pool(name="ps_s", bufs=1, space="PSUM"))
        ps_e = ctx.enter_context(tc.tile_pool(name="ps_e", bufs=2, space="PSUM"))
        ps_z = ctx.enter_context(tc.tile_pool(name="ps_z", bufs=1, space="PSUM"))
        ps_b = ctx.enter_context(tc.tile_pool(name="ps_b", bufs=1, space="PSUM"))
        ps_t = ctx.enter_context(tc.tile_pool(name="ps_t", bufs=1, space="PSUM"))
        dram = ctx.enter_context(tc.tile_pool(name="dram", bufs=1, space="DRAM"))

        # --- dummy collective: absorbs the rank-entry barrier + start skew
        # while input DMA / iter-0 compute proceed underneath.
        dmy = singles.tile([1, 1], f32, tag="dmy")
        nc.vector.memset(dmy, 0.0)
        dci = dram.tile([1, 1], f32, tag="dmy_i", name="dmy_i")
        dco = dram.tile([1, 1], f32, tag="dmy_o", name="dmy_o")
        nc.sync.dma_start(out=dci[:], in_=dmy)
        nc.gpsimd.collective_compute(
            "AllReduce", ALU.add,
            replica_groups=rgroups, ins=[dci.opt()], outs=[dco.opt()],
        )

        # --- input DMAs (bf16 blob first: it carries everything iter-0 needs)
        sb_b = singles.tile([128, DWB], bf16, tag="data_b")
        nc.sync.dma_start(out=sb_b, in_=data_b[:])
        sb_f = singles.tile([128, DWF], f32, tag="data_f")
        nc.sync.dma_start(out=sb_f, in_=data_f[:])
        sb_uTb = sb_b[:, o_uT:o_uT + NCHUNK * B]
        sb_Wtb = sb_b[:, o_Wtb:o_Wtb + NCHUNK * CD]
        sb_sel = sb_b[:, o_sel:o_sel + NCHUNK * 128]
        sb_Wt = sb_f[:, o_Wt:o_Wt + NCHUNK * CD]
        sb_mask = sb_f[:, o_mk:o_mk + 16]

        # per-iteration collective buffers
        cc = []
        for it in range(NITER):
            w = 2 * CD if it == 0 else 3 * CD
            ci = dram.tile([128, w], bf16, tag=f"cc_in{it}", name=f"cc_in{it}")
            co = dram.tile([128, w], bf16, tag=f"cc_out{it}", name=f"cc_out{it}")
            cc.append((ci, co, w))

        # --- iteration 0 s-matmuls straight off the DMA'd bf16 inputs
        st = [ps_s.tile([128, CD], f32, tag=f"s{bh}", name=f"s{bh}") for bh in range(2)]
        for k in range(NCHUNK):
            for bh in range(2):
                nc.tensor.matmul(
                    st[bh],
                    sb_uTb[:, k * B + bh * 128: k * B + (bh + 1) * 128],
                    sb_Wtb[:, k * CD:(k + 1) * CD],
                    start=(k == 0), stop=(k == NCHUNK - 1),
                )
        ci0, co0, w0 = cc[0]
        stage = stpool.tile([128, w0], bf16, tag="stage", name="stage0")
        for bh in range(2):
            nc.vector.tensor_copy(out=stage[:, bh * CD:(bh + 1) * CD], in_=st[bh])
        nc.sync.dma_start(out=ci0[:], in_=stage)
        nc.gpsimd.collective_compute(
            "AllReduce", ALU.add,
            replica_groups=rgroups, ins=[ci0.opt()], outs=[co0.opt()],
        )
        red0 = stpool.tile([128, w0], bf16, tag="red", name="red0")
        nc.sync.dma_start(out=red0, in_=co0[:])

        # --- setup, scheduled after AR0's trigger -> hidden under the AR
        sb_ones = singles.tile([128, 128], bf16, tag="ones")
        nc.vector.memset(sb_ones, 1.0)

        # Wd[(rp,i), (k,c)] = (1/B) * sum_d Wt
        sb_Wd = singles.tile([128, NCHUNK * C], f32, tag="Wd")
        for k in range(NCHUNK):
            nc.vector.reduce_sum(
                out=sb_Wd[:, k * C:(k + 1) * C],
                in_=sb_Wt[:, k * CD:(k + 1) * CD].rearrange("p (c d) -> p c d", d=D),
                axis=mybir.AxisListType.X,
            )
        nc.vector.tensor_scalar_mul(sb_Wd, sb_Wd, 1.0 / B)

        # Wdbd[p, (k, c, rp)] = Wd[p, (k,c)] * mask2[p, rp]   (block-diagonal)
        sb_Wdbd = singles.tile([128, NCHUNK * CD], bf16, tag="Wdbd")
        wd_b = bass.AP(
            tensor=sb_Wd.tensor, offset=sb_Wd.offset,
            ap=[sb_Wd.ap[0], [C, NCHUNK], [1, C], [0, 16]],
        )
        mk_b = bass.AP(
            tensor=sb_mask.tensor, offset=sb_mask.offset,
            ap=[sb_mask.ap[0], [0, NCHUNK], [0, C], [1, 16]],
        )
        nc.vector.tensor_mul(
            sb_Wdbd.rearrange("p (k c rp) -> p k c rp", c=C, rp=16), wd_b, mk_b
        )

        # t[b, (c, k, rp)] = sum_i Wd[(rp,i),(k,c)] u[b, r(k,rp), i]
        sb_t = [singles.tile([128, C * RL], bf16, tag=f"t{bh}", name=f"t{bh}")
                for bh in range(2)]
        t_r = [sb_t[bh].rearrange("p (c k rp) -> p k c rp", k=NCHUNK, rp=16)
               for bh in range(2)]
        for k in range(NCHUNK):
            for bh in range(2):
                pt = ps_t.tile([128, CD], f32, tag="pt", name="pt")
                nc.tensor.matmul(
                    pt,
                    sb_uTb[:, k * B + bh * 128: k * B + (bh + 1) * 128],
                    sb_Wdbd[:, k * CD:(k + 1) * CD],
                    start=True, stop=True,
                )
                nc.vector.tensor_copy(
                    out=t_r[bh][:, k],
                    in_=pt.rearrange("p (c rp) -> p c rp", rp=16),
                )

        # b_ij: main [128=(k<8,rp), (c,d)] f32 + tail [16=rp(k=8), (c,d)] f32
        b_main = singles.tile([128, CD], f32, tag="bm")
        nc.vector.memset(b_main, 0.0)
        b_tail = singles.tile([16, CD], f32, tag="bt")
        nc.vector.memset(b_tail, 0.0)
        E_main = singles.tile([128, CD], bf16, tag="Em")
        E_tail = singles.tile([16, CD], bf16, tag="Et")
        sb_vb = singles.tile([128, 2 * CD], bf16, tag="vb")
        sb_vf = singles.tile([128, 2 * CD], f32, tag="vf")

        for it in range(NITER):
            # ---- v = y|y| / (den) from the AllReduce result of iteration it
            red = red0 if it == 0 else red
            y = red[:, 0:2 * CD]
            sq = work.tile([128, 2 * CD], f32, tag="sq")
            nc.vector.tensor_mul(sq, y, y)
            negy = work.tile([128, 2 * CD], bf16, tag="negy")
            nc.vector.tensor_scalar_mul(negy, y, -1.0)
            absy = work.tile([128, 2 * CD], bf16, tag="absy")
            nc.vector.tensor_tensor(out=absy, in0=y, in1=negy, op=ALU.max)
            den = work.tile([128, 2 * CD], f32, tag="den")
            if it == 0:
                nc.vector.tensor_scalar_add(den, sq, float(R) * float(R))
            else:
                zr = red[:, 2 * CD:3 * CD]
                z2 = work.tile([128, CD], f32, tag="z2")
                nc.vector.tensor_mul(z2, zr, zr)
                z2b = bass.AP(
                    tensor=z2.tensor, offset=z2.offset,
                    ap=[z2.ap[0], [0, 2], [1, CD]],
                )
                nc.vector.tensor_add(
                    den.rearrange("p (h f) -> p h f", f=CD),
                    sq.rearrange("p (h f) -> p h f", f=CD),
                    z2b,
                )
            rec = work.tile([128, 2 * CD], f32, tag="rec")
            nc.vector.reciprocal_approx_fast(out=rec, in_=den)
            num = work.tile([128, 2 * CD], f32, tag="num")
            nc.vector.tensor_mul(num, y, absy)
            if it < NITER - 1:
                nc.vector.tensor_mul(sb_vb, num, rec)
            else:
                nc.vector.tensor_mul(sb_vf, num, rec)
                for bh in range(2):
                    nc.sync.dma_start(
                        out=out_d[bh * 128:(bh + 1) * 128, :],
                        in_=sb_vf[:, bh * CD:(bh + 1) * CD],
                    )
                break

            # ---- b-update: 20 main (M=128) + 20 tail (M=16) matmuls
            pb = ps_b.tile([128, CD], f32, tag="pb", name="pb")
            pbt = ps_b.tile([16, CD], f32, tag="pbt", name="pbt")
            for c in range(C):
                for bh in range(2):
                    nc.tensor.matmul(
                        pb[:, c * D:(c + 1) * D],
                        sb_t[bh][:, c * RL: c * RL + 128],
                        sb_vb[:, bh * CD + c * D: bh * CD + (c + 1) * D],
                        start=(bh == 0), stop=(bh == 1),
                    )
            for c in range(C):
                for bh in range(2):
                    nc.tensor.matmul(
                        pbt[:, c * D:(c + 1) * D],
                        sb_t[bh][:, c * RL + 128: (c + 1) * RL],
                        sb_vb[:, bh * CD + c * D: bh * CD + (c + 1) * D],
                        start=(bh == 0), stop=(bh == 1),
                    )
            nc.vector.tensor_add(b_main, b_main, pb)
            nc.vector.tensor_add(b_tail, b_tail, pbt)

            # ---- next iteration: E = exp(b), Z-partial, Weff, s-matmuls
            nc.scalar.activation(out=E_main, in_=b_main, func=AF.Exp)
            nc.scalar.activation(out=E_tail, in_=b_tail, func=AF.Exp)
            pz = ps_z.tile([128, CD], f32, tag="pz", name="pz")
            nc.tensor.matmul(pz, sb_ones, E_main, start=True, stop=False)
            nc.tensor.matmul(pz, sb_ones[0:16, :], E_tail, start=False, stop=True)

            ci, co, w = cc[it + 1]
            st = [ps_s.tile([128, CD], f32, tag=f"s{bh}", name=f"s{bh}")
                  for bh in range(2)]
            for k in range(NCHUNK):
                pe = ps_e.tile([128, CD], f32, tag="pe", name="pe")
                if k < KMAIN:
                    nc.tensor.matmul(
                        pe, sb_sel[:, k * 128:(k + 1) * 128], E_main,
                        start=True, stop=True,
                    )
                else:
                    nc.tensor.matmul(
                        pe, sb_sel[0:16, KMAIN * 128:(KMAIN + 1) * 128], E_tail,
                        start=True, stop=True,
                    )
                rhs = wcpool.tile([128, CD], bf16, tag="wc")
                nc.vector.tensor_mul(rhs, sb_Wt[:, k * CD:(k + 1) * CD], pe)
                for bh in range(2):
                    nc.tensor.matmul(
                        st[bh],
                        sb_uTb[:, k * B + bh * 128: k * B + (bh + 1) * 128],
                        rhs,
                        start=(k == 0), stop=(k == NCHUNK - 1),
                    )

            stage = stpool.tile([128, w], bf16, tag="stage", name=f"stage{it+1}")
            for bh in range(2):
                nc.vector.tensor_copy(out=stage[:, bh * CD:(bh + 1) * CD], in_=st[bh])
            nc.vector.tensor_copy(out=stage[:, 2 * CD:3 * CD], in_=pz)
            nc.sync.dma_start(out=ci[:], in_=stage)
            nc.gpsimd.collective_compute(
                "AllReduce", ALU.add,
                replica_groups=rgroups, ins=[ci.opt()], outs=[co.opt()],
            )
            red = stpool.tile([128, w], bf16, tag="red", name=f"red{it+1}")
            nc.sync.dma_start(out=red, in_=co[:])

    nc.compile()
    return nc


def _host_inputs(u, W):
    """Pure permutation + dtype cast host prep: per-core (r,i)-major layouts."""
    import concourse.mybir as mybir

    bf16 = mybir.dt.np(mybir.dt.bfloat16)
    u = np.ascontiguousarray(u, dtype=np.float32)
    W = np.ascontiguousarray(W, dtype=np.float32)

    o_uT, o_Wtb, o_sel = 0, NCHUNK * B, NCHUNK * B + NCHUNK * CD
    DWB = o_sel + NCHUNK * 128
    o_Wt, o_mk = 0, NCHUNK * CD
    DWF = o_mk + 16

    sel = np.zeros((128, NCHUNK * 128), dtype=np.float32)
    for k in range(KMAIN):
        for rp in range(16):
            sel[k * 16 + rp, k * 128 + rp * 8:(k * 128 + rp * 8) + 8] = 1.0
    for rp in range(16):
        sel[rp, KMAIN * 128 + rp * 8: KMAIN * 128 + rp * 8 + 8] = 1.0
    mask2 = np.zeros((128, 16), dtype=np.float32)
    for p in range(128):
        mask2[p, p // 8] = 1.0

    in_maps = []
    for ci in range(NCORES):
        rs = ci * RL
        usl = u[:, rs:rs + RL, :].reshape(B, RL * I).T          # (1152, 256)
        uTd = usl.reshape(NCHUNK, 128, B).transpose(1, 0, 2).reshape(128, NCHUNK * B)
        wsl = W[rs:rs + RL].transpose(0, 3, 1, 2).reshape(RL * I, CD)
        Wtd = wsl.reshape(NCHUNK, 128, CD).transpose(1, 0, 2).reshape(128, NCHUNK * CD)
        db = np.zeros((128, DWB), dtype=np.float32)
        db[:, o_uT:o_uT + NCHUNK * B] = uTd
        db[:, o_Wtb:o_Wtb + NCHUNK * CD] = Wtd
        db[:, o_sel:o_sel + NCHUNK * 128] = sel
        df = np.zeros((128, DWF), dtype=np.float32)
        df[:, o_Wt:o_Wt + NCHUNK * CD] = Wtd
        df[:, o_mk:o_mk + 16] = mask2
        in_maps.append({"data_b": db.astype(bf16), "data_f": df})
    return in_maps


def _install_profile_hook():
    """Recreate the missing antenv.axon_hooks NTFF-profile hook (dev only)."""
    import contextlib
    import ctypes
    import sys
    import types

    try:
        from antenv.axon_hooks import get_axon_ntff_profile_hook  # noqa: F401
        return
    except ImportError:
        pass

    mod = types.ModuleType("antenv.axon_hooks")
    holder = {}
    mod.set_axon_ntff_profile_hook = lambda h: holder.__setitem__("h", h)
    mod.get_axon_ntff_profile_hook = lambda: holder.get("h")
    import antenv

    sys.modules["antenv.axon_hooks"] = mod
    antenv.axon_hooks = mod

    so_path = "/opt/axon/libaxon_pjrt.so"
    lib = ctypes.CDLL(so_path)
    if not hasattr(lib, "axon_start_nrt_profile"):
        return
    lib.axon_start_nrt_profile.argtypes = [
        ctypes.POINTER(ctypes.c_int64),
        ctypes.c_size_t,
    ]
    lib.axon_start_nrt_profile.restype = ctypes.c_int64
    lib.axon_stop_nrt_profile.argtypes = [ctypes.c_char_p]
    lib.axon_stop_nrt_profile.restype = ctypes.c_int64

    @contextlib.contextmanager
    def _hook(output_dir, device_ids):
        import jax

        jax.devices()
        if device_ids:
            ids = (ctypes.c_int64 * len(device_ids))(*device_ids)
            rc = lib.axon_start_nrt_profile(ids, len(device_ids))
        else:
            rc = lib.axon_start_nrt_profile(None, 0)
        if rc != 0:
            raise RuntimeError(f"axon_start_nrt_profile rc={rc}")
        try:
            yield
        finally:
            n = lib.axon_stop_nrt_profile(str(output_dir).encode())
            print(f"profile: {n} file(s) written to {output_dir}")

    mod.set_axon_ntff_profile_hook(_hook)

    import concourse.bass_utils as bu

    bu.upload_artifacts = lambda tmpdir: f"local:{tmpdir}"


def kernel(u, W):
    from concourse.bass_utils import run_bass_kernel_spmd

    if os.environ.get("KERNEL_TRACE", "0") == "1":
        _install_profile_hook()
    if "nc" not in _CACHE:
        _CACHE["nc"] = _build_program()
    nc = _CACHE["nc"]
    in_maps = _host_inputs(u, W)
    trace = os.environ.get("KERNEL_TRACE", "0") == "1"
    res = run_bass_kernel_spmd(
        nc, in_maps, core_ids=list(range(NCORES)), trace=trace
    )
    _CACHE["last_result"] = res
    return np.asarray(res.results[0]["out"]).reshape(B, C, D)


# revision 13
# speedup vs baseline: 1.7745x; 1.7745x over previous
"""DigitCaps dynamic-routing kernel for 8 TRN2 NeuronCores (v2).

Math refactor (u_hat is NEVER materialized - it would be 189 MB):
  u_hat[b,r,c,d] = sum_i W[r,c,d,i] * u[b,r,i]
  softmax over r without max-subtraction (b_ij values are O(1)):
      c_ij[r,c,d] = exp(b[r,c,d]) / Z[c,d],  Z = sum_r exp(b)
  s[b,c,d]  = (sum_{r,i} (exp(b) * W)[r,c,d,i] u[b,r,i]) / Z[c,d]
  v = squash(s) = s|s|/(1+s^2)  (eps dropped; rel err ~1e-4)
    = y|y| / (Z^2 + y^2) with y the UN-normalized s  (Z-division folded in)
  b += (1/B) sum_b t[b,r,c] v[b,c,d],  t[b,r,c] = sum_i (sum_d W)[r,c,i] u[b,r,i]
       (t is iteration-invariant -> computed once, hidden under AllReduce 0)

Sharding: routes (R=1152) split across 8 cores (144 each). Per iteration one
fused bf16 AllReduce carries the partial s' (B x C*D) and partial Z.

v2 changes vs v1:
  - bf16 on the wire (uT, Wtb, sel) + bf16 AllReduce payloads
  - b_ij layout [(k,rp)=128, (c,d)] + [16-route tail] so each b-update
    (c,bh) pair is ONE M=128 matmul: 40 matmuls/update instead of 60
  - t stored [b, (c,k,rp)] so b-update lhsT slices are contiguous
  - squash via abs_max + reciprocal_approx_fast (no Sqrt -> no ACT table
    swaps; approx recip is ~5x faster than nc.vector.reciprocal)
  - dummy 4-byte AllReduce first thing to absorb the rank-entry barrier
  - program order puts iter-0 s-matmuls before all setup so AR0 starts ASAP
"""

import os
import numpy as np

B, R, C, D, I = 256, 1152, 10, 16, 8
CD = C * D                 # 160
NCORES = 8
RL = R // NCORES           # 144 routes per core
NCHUNK = RL * I // 128     # 9 K-chunks of 128
KMAIN = 8                  # chunks whose routes live in b_main (8*16=128)
NITER = 3

_CACHE = {}


def _build_program():
    from contextlib import ExitStack

    import concourse.bass as bass
    import concourse.bacc as bacc
    import concourse.mybir as mybir
    import concourse.tile as tile

    f32 = mybir.dt.float32
    bf16 = mybir.dt.bfloat16
    AF = mybir.ActivationFunctionType
    ALU = mybir.AluOpType

    nc = bacc.Bacc(None, num_devices=NCORES)

    # bf16 blob: uTb | Wtb | sel   (cols 2304 | 1440 | 1152)
    o_uT, o_Wtb, o_sel = 0, NCHUNK * B, NCHUNK * B + NCHUNK * CD
    DWB = o_sel + NCHUNK * 128
    # f32 blob: Wt | mask2        (cols 1440 | 16)
    o_Wt, o_mk = 0, NCHUNK * CD
    DWF = o_mk + 16
    data_b = nc.declare_dram_parameter("data_b", [128, DWB], bf16, isOutput=False)
    data_f = nc.declare_dram_parameter("data_f", [128, DWF], f32, isOutput=False)
    out_d = nc.declare_dram_parameter("out", [B, CD], f32, isOutput=True)

    rgroups = [list(range(NCORES))]

    with tile.TileContext(nc) as tc, ExitStack() as ctx:
        singles = ctx.enter_context(tc.tile_pool(name="singles", bufs=1))
        wcpool = ctx.enter_context(tc.tile_pool(name="wc", bufs=3))
        stpool = ctx.enter_context(tc.tile_pool(name="stage", bufs=2))
        work = ctx.enter_context(tc.tile_pool(name="work", bufs=2))
        ps_s = ctx.enter_context(tc.tile_pool(name="ps_s", bufs=1, space="PSUM"))
        ps_e = ctx.enter_context(tc.tile_pool(name="ps_e", bufs=2, space="PSUM"))
        ps_z = ctx.enter_context(tc.tile_pool(name="ps_z", bufs=1, space="PSUM"))
        ps_b = ctx.enter_context(tc.tile_pool(name="ps_b", bufs=1, space="PSUM"))
        ps_t = ctx.enter_context(tc.tile_pool(name="ps_t", bufs=1, space="PSUM"))
        dram = ctx.enter_context(tc.tile_pool(name="dram", bufs=1, space="DRAM"))

        # --- input DMAs (bf16 blob first: it carries everything iter-0 needs)
        sb_b = singles.tile([128, DWB], bf16, tag="data_b")
        nc.sync.dma_start(out=sb_b, in_=data_b[:])
        sb_f = singles.tile([128, DWF], f32, tag="data_f")
        nc.sync.dma_start(out=sb_f, in_=data_f[:])
        sb_uTb = sb_b[:, o_uT:o_uT + NCHUNK * B]
        sb_Wtb = sb_b[:, o_Wtb:o_Wtb + NCHUNK * CD]
        sb_sel = sb_b[:, o_sel:o_sel + NCHUNK * 128]
        sb_Wt = sb_f[:, o_Wt:o_Wt + NCHUNK * CD]
        sb_mask = sb_f[:, o_mk:o_mk + 16]

        # per-iteration collective buffers
        cc = []
        for it in range(NITER):
            w = 2 * CD if it == 0 else 3 * CD
            ci = dram.tile([128, w], bf16, tag=f"cc_in{it}", name=f"cc_in{it}")
            co = dram.tile([128, w], bf16, tag=f"cc_out{it}", name=f"cc_out{it}")
            cc.append((ci, co, w))

        # --- iteration 0 s-matmuls straight off the DMA'd bf16 inputs
        # (separate PSUM tiles per bh: start=True clears has_written for the
        # whole BANK, so interleaved accumulation chains must not share one)
        st = [ps_s.tile([128, CD], f32, tag=f"s{bh}", name=f"s0_{bh}")
              for bh in range(2)]
        for k in range(NCHUNK):
            for bh in range(2):
                nc.tensor.matmul(
                    st[bh],
                    sb_uTb[:, k * B + bh * 128: k * B + (bh + 1) * 128],
                    sb_Wtb[:, k * CD:(k + 1) * CD],
                    start=(k == 0), stop=(k == NCHUNK - 1),
                )
        ci0, co0, w0 = cc[0]
        stage = stpool.tile([128, w0], bf16, tag="stage", name="stage0")
        for bh in range(2):
            nc.vector.tensor_copy(out=stage[:, bh * CD:(bh + 1) * CD], in_=st[bh])
        nc.sync.dma_start(out=ci0[:], in_=stage)
        nc.gpsimd.collective_compute(
            "AllReduce", ALU.add,
            replica_groups=rgroups, ins=[ci0.opt()], outs=[co0.opt()],
        )
        red0 = stpool.tile([128, w0], bf16, tag="red", name="red0")
        nc.sync.dma_start(out=red0, in_=co0[:])

        # --- setup, scheduled after AR0's trigger -> hidden under the AR
        sb_ones = singles.tile([128, 128], bf16, tag="ones")
        nc.vector.memset(sb_ones, 1.0)

        # Wd[(rp,i), (k,c)] = (1/B) * sum_d Wt
        sb_Wd = singles.tile([128, NCHUNK * C], f32, tag="Wd")
        for k in range(NCHUNK):
            nc.vector.reduce_sum(
                out=sb_Wd[:, k * C:(k + 1) * C],
                in_=sb_Wt[:, k * CD:(k + 1) * CD].rearrange("p (c d) -> p c d", d=D),
                axis=mybir.AxisListType.X,
            )
        nc.vector.tensor_scalar_mul(sb_Wd, sb_Wd, 1.0 / B)

        # Wdbd[p, (k, c, rp)] = Wd[p, (k,c)] * mask2[p, rp]   (block-diagonal)
        sb_Wdbd = singles.tile([128, NCHUNK * CD], bf16, tag="Wdbd")
        wd_b = bass.AP(
            tensor=sb_Wd.tensor, offset=sb_Wd.offset,
            ap=[sb_Wd.ap[0], [C, NCHUNK], [1, C], [0, 16]],
        )
        mk_b = bass.AP(
            tensor=sb_mask.tensor, offset=sb_mask.offset,
            ap=[sb_mask.ap[0], [0, NCHUNK], [0, C], [1, 16]],
        )
        nc.vector.tensor_mul(
            sb_Wdbd.rearrange("p (k c rp) -> p k c rp", c=C, rp=16), wd_b, mk_b
        )

        # t[b, (c, k, rp)] = sum_i Wd[(rp,i),(k,c)] u[b, r(k,rp), i]
        sb_t = [singles.tile([128, C * RL], bf16, tag=f"t{bh}", name=f"t{bh}")
                for bh in range(2)]
        t_r = [sb_t[bh].rearrange("p (c k rp) -> p k c rp", k=NCHUNK, rp=16)
               for bh in range(2)]
        for k in range(NCHUNK):
            for bh in range(2):
                pt = ps_t.tile([128, CD], f32, tag="pt", name="pt")
                nc.tensor.matmul(
                    pt,
                    sb_uTb[:, k * B + bh * 128: k * B + (bh + 1) * 128],
                    sb_Wdbd[:, k * CD:(k + 1) * CD],
                    start=True, stop=True,
                )
                nc.vector.tensor_copy(
                    out=t_r[bh][:, k],
                    in_=pt.rearrange("p (c rp) -> p c rp", rp=16),
                )

        # b_ij: main [128=(k<8,rp), (c,d)] f32 + tail [16=rp(k=8), (c,d)] f32
        b_main = singles.tile([128, CD], f32, tag="bm")
        nc.vector.memset(b_main, 0.0)
        b_tail = singles.tile([16, CD], f32, tag="bt")
        nc.vector.memset(b_tail, 0.0)
        E_main = singles.tile([128, CD], bf16, tag="Em")
        E_tail = singles.tile([16, CD], bf16, tag="Et")
        sb_vb = singles.tile([128, 2 * CD], bf16, tag="vb")
        sb_vf = singles.tile([128, 2 * CD], f32, tag="vf")

        for it in range(NITER):
            # ---- v = y|y| / (den) from the AllReduce result of iteration it
            red = red0 if it == 0 else red
            y = red[:, 0:2 * CD]
            sq = work.tile([128, 2 * CD], f32, tag="sq")
            nc.vector.tensor_mul(sq, y, y)
            negy = work.tile([128, 2 * CD], bf16, tag="negy")
            nc.vector.tensor_scalar_mul(negy, y, -1.0)
            absy = work.tile([128, 2 * CD], bf16, tag="absy")
            nc.vector.tensor_tensor(out=absy, in0=y, in1=negy, op=ALU.max)
            den = work.tile([128, 2 * CD], f32, tag="den")
            if it == 0:
                nc.vector.tensor_scalar_add(den, sq, float(R) * float(R))
            else:
                zr = red[:, 2 * CD:3 * CD]
                z2 = work.tile([128, CD], f32, tag="z2")
                nc.vector.tensor_mul(z2, zr, zr)
                z2b = bass.AP(
                    tensor=z2.tensor, offset=z2.offset,
                    ap=[z2.ap[0], [0, 2], [1, CD]],
                )
                nc.vector.tensor_add(
                    den.rearrange("p (h f) -> p h f", f=CD),
                    sq.rearrange("p (h f) -> p h f", f=CD),
                    z2b,
                )
            rec = work.tile([128, 2 * CD], f32, tag="rec")
            nc.vector.reciprocal_approx_fast(out=rec, in_=den)
            num = work.tile([128, 2 * CD], f32, tag="num")
            nc.vector.tensor_mul(num, y, absy)
            if it < NITER - 1:
                nc.vector.tensor_mul(sb_vb, num, rec)
            else:
                nc.vector.tensor_mul(sb_vf, num, rec)
                for bh in range(2):
                    nc.sync.dma_start(
                        out=out_d[bh * 128:(bh + 1) * 128, :],
                        in_=sb_vf[:, bh * CD:(bh + 1) * CD],
                    )
                break

            # ---- b-update: 20 main (M=128) + 20 tail (M=16) matmuls
            pb = ps_b.tile([128, CD], f32, tag="pb", name="pb")
            pbt = ps_b.tile([16, CD], f32, tag="pbt", name="pbt")
            for c in range(C):
                for bh in range(2):
                    nc.tensor.matmul(
                        pb[:, c * D:(c + 1) * D],
                        sb_t[bh][:, c * RL: c * RL + 128],
                        sb_vb[:, bh * CD + c * D: bh * CD + (c + 1) * D],
                        start=(bh == 0), stop=(bh == 1),
                    )
            for c in range(C):
                for bh in range(2):
                    nc.tensor.matmul(
                        pbt[:, c * D:(c + 1) * D],
                        sb_t[bh][:, c * RL + 128: (c + 1) * RL],
                        sb_vb[:, bh * CD + c * D: bh * CD + (c + 1) * D],
                        start=(bh == 0), stop=(bh == 1),
                    )
            nc.vector.tensor_add(b_main, b_main, pb)
            nc.vector.tensor_add(b_tail, b_tail, pbt)

            # ---- next iteration: E = exp(b), Z-partial, Weff, s-matmuls
            nc.scalar.activation(out=E_main, in_=b_main, func=AF.Exp)
            nc.scalar.activation(out=E_tail, in_=b_tail, func=AF.Exp)
            pz = ps_z.tile([128, CD], f32, tag="pz", name="pz")
            nc.tensor.matmul(pz, sb_ones, E_main, start=True, stop=False)
            nc.tensor.matmul(pz, sb_ones[0:16, :], E_tail, start=False, stop=True)

            ci, co, w = cc[it + 1]
            stage = stpool.tile([128, w], bf16, tag="stage", name=f"stage{it+1}")
            nc.scalar.activation(
                out=stage[:, 2 * CD:3 * CD], in_=pz, func=AF.Copy
            )
            st = [ps_s.tile([128, CD], f32, tag=f"s{bh}", name=f"s{it+1}_{bh}")
                  for bh in range(2)]

            def emit_pe(g):
                pe = ps_e.tile([128, 3 * CD], f32, tag="pe", name="pe")
                for j in range(3):
                    k = 3 * g + j
                    if k < KMAIN:
                        nc.tensor.matmul(
                            pe[:, j * CD:(j + 1) * CD],
                            sb_sel[:, k * 128:(k + 1) * 128], E_main,
                            start=True, stop=True,
                        )
                    else:
                        nc.tensor.matmul(
                            pe[:, j * CD:(j + 1) * CD],
                            sb_sel[0:16, KMAIN * 128:(KMAIN + 1) * 128], E_tail,
                            start=True, stop=True,
                        )
                return pe

            def emit_st(g, pe):
                rhs = wcpool.tile([128, 3 * CD], bf16, tag="wc")
                nc.vector.tensor_mul(
                    rhs, sb_Wt[:, 3 * g * CD:(3 * g + 3) * CD], pe
                )
                for j in range(3):
                    k = 3 * g + j
                    for bh in range(2):
                        nc.tensor.matmul(
                            st[bh],
                            sb_uTb[:, k * B + bh * 128: k * B + (bh + 1) * 128],
                            rhs[:, j * CD:(j + 1) * CD],
                            start=(k == 0), stop=(k == NCHUNK - 1),
                        )

            pe0 = emit_pe(0)
            pe1 = emit_pe(1)
            emit_st(0, pe0)
            pe2 = emit_pe(2)
            emit_st(1, pe1)
            emit_st(2, pe2)

            for bh in range(2):
                nc.vector.tensor_copy(
                    out=stage[:, bh * CD:(bh + 1) * CD], in_=st[bh]
                )
            nc.sync.dma_start(out=ci[:], in_=stage)
            nc.gpsimd.collective_compute(
                "AllReduce", ALU.add,
                replica_groups=rgroups, ins=[ci.opt()], outs=[co.opt()],
            )
            red = stpool.tile([128, w], bf16, tag="red", name=f"red{it+1}")
            nc.sync.dma_start(out=red, in_=co[:])

    nc.compile()
    return nc


def _host_inputs(u, W):
    """Pure permutation + dtype cast host prep: per-core (r,i)-major layouts."""
    import concourse.mybir as mybir

    bf16 = mybir.dt.np(mybir.dt.bfloat16)
    u = np.ascontiguousarray(u, dtype=np.float32)
    W = np.ascontiguousarray(W, dtype=np.float32)

    o_uT, o_Wtb, o_sel = 0, NCHUNK * B, NCHUNK * B + NCHUNK * CD
    DWB = o_sel + NCHUNK * 128
    o_Wt, o_mk = 0, NCHUNK * CD
    DWF = o_mk + 16

    sel = np.zeros((128, NCHUNK * 128), dtype=np.float32)
    for k in range(KMAIN):
        for rp in range(16):
            sel[k * 16 + rp, k * 128 + rp * 8:(k * 128 + rp * 8) + 8] = 1.0
    for rp in range(16):
        sel[rp, KMAIN * 128 + rp * 8: KMAIN * 128 + rp * 8 + 8] = 1.0
    mask2 = np.zeros((128, 16), dtype=np.float32)
    for p in range(128):
        mask2[p, p // 8] = 1.0

    in_maps = []
    for ci in range(NCORES):
        rs = ci * RL
        usl = u[:, rs:rs + RL, :].reshape(B, RL * I).T          # (1152, 256)
        uTd = usl.reshape(NCHUNK, 128, B).transpose(1, 0, 2).reshape(128, NCHUNK * B)
        wsl = W[rs:rs + RL].transpose(0, 3, 1, 2).reshape(RL * I, CD)
        Wtd = wsl.reshape(NCHUNK, 128, CD).transpose(1, 0, 2).reshape(128, NCHUNK * CD)
        db = np.zeros((128, DWB), dtype=np.float32)
        db[:, o_uT:o_uT + NCHUNK * B] = uTd
        db[:, o_Wtb:o_Wtb + NCHUNK * CD] = Wtd
        db[:, o_sel:o_sel + NCHUNK * 128] = sel
        df = np.zeros((128, DWF), dtype=np.float32)
        df[:, o_Wt:o_Wt + NCHUNK * CD] = Wtd
        df[:, o_mk:o_mk + 16] = mask2
        in_maps.append({"data_b": db.astype(bf16), "data_f": df})
    return in_maps


def _install_profile_hook():
    """Recreate the missing antenv.axon_hooks NTFF-profile hook (dev only)."""
    import contextlib
    import ctypes
    import sys
    import types

    try:
        from antenv.axon_hooks import get_axon_ntff_profile_hook  # noqa: F401
        return
    except ImportError:
        pass

    mod = types.ModuleType("antenv.axon_hooks")
    holder = {}
    mod.set_axon_ntff_profile_hook = lambda h: holder.__setitem__("h", h)
    mod.get_axon_ntff_profile_hook = lambda: holder.get("h")
    import antenv

    sys.modules["antenv.axon_hooks"] = mod
    antenv.axon_hooks = mod

    so_path = "/opt/axon/libaxon_pjrt.so"
    lib = ctypes.CDLL(so_path)
    if not hasattr(lib, "axon_start_nrt_profile"):
        return
    lib.axon_start_nrt_profile.argtypes = [
        ctypes.POINTER(ctypes.c_int64),
        ctypes.c_size_t,
    ]
    lib.axon_start_nrt_profile.restype = ctypes.c_int64
    lib.axon_stop_nrt_profile.argtypes = [ctypes.c_char_p]
    lib.axon_stop_nrt_profile.restype = ctypes.c_int64

    @contextlib.contextmanager
    def _hook(output_dir, device_ids):
        import jax

        jax.devices()
        if device_ids:
            ids = (ctypes.c_int64 * len(device_ids))(*device_ids)
            rc = lib.axon_start_nrt_profile(ids, len(device_ids))
        else:
            rc = lib.axon_start_nrt_profile(None, 0)
        if rc != 0:
            raise RuntimeError(f"axon_start_nrt_profile rc={rc}")
        try:
            yield
        finally:
            n = lib.axon_stop_nrt_profile(str(output_dir).encode())
            print(f"profile: {n} file(s) written to {output_dir}")

    mod.set_axon_ntff_profile_hook(_hook)

    import concourse.bass_utils as bu

    bu.upload_artifacts = lambda tmpdir: f"local:{tmpdir}"


def kernel(u, W):
    from concourse.bass_utils import run_bass_kernel_spmd

    if os.environ.get("KERNEL_TRACE", "0") == "1":
        _install_profile_hook()
    if "nc" not in _CACHE:
        _CACHE["nc"] = _build_program()
    nc = _CACHE["nc"]
    in_maps = _host_inputs(u, W)
    trace = os.environ.get("KERNEL_TRACE", "0") == "1"
    res = run_bass_kernel_spmd(
        nc, in_maps, core_ids=list(range(NCORES)), trace=trace
    )
    _CACHE["last_result"] = res
    return np.asarray(res.results[0]["out"]).reshape(B, C, D)


# revision 14
# speedup vs baseline: 1.9904x; 1.1217x over previous
"""DigitCaps dynamic-routing kernel for 8 TRN2 NeuronCores (v2).

Math refactor (u_hat is NEVER materialized - it would be 189 MB):
  u_hat[b,r,c,d] = sum_i W[r,c,d,i] * u[b,r,i]
  softmax over r without max-subtraction (b_ij values are O(1)):
      c_ij[r,c,d] = exp(b[r,c,d]) / Z[c,d],  Z = sum_r exp(b)
  s[b,c,d]  = (sum_{r,i} (exp(b) * W)[r,c,d,i] u[b,r,i]) / Z[c,d]
  v = squash(s) = s|s|/(1+s^2)  (eps dropped; rel err ~1e-4)
    = y|y| / (Z^2 + y^2) with y the UN-normalized s  (Z-division folded in)
  b += (1/B) sum_b t[b,r,c] v[b,c,d],  t[b,r,c] = sum_i (sum_d W)[r,c,i] u[b,r,i]
       (t is iteration-invariant -> computed once, hidden under AllReduce 0)

Sharding: routes (R=1152) split across 8 cores (144 each). Per iteration one
fused bf16 AllReduce carries the partial s' (B x C*D) and partial Z.

v2 changes vs v1:
  - bf16 on the wire (uT, Wtb, sel) + bf16 AllReduce payloads
  - b_ij layout [(k,rp)=128, (c,d)] + [16-route tail] so each b-update
    (c,bh) pair is ONE M=128 matmul: 40 matmuls/update instead of 60
  - t stored [b, (c,k,rp)] so b-update lhsT slices are contiguous
  - squash via abs_max + reciprocal_approx_fast (no Sqrt -> no ACT table
    swaps; approx recip is ~5x faster than nc.vector.reciprocal)
  - dummy 4-byte AllReduce first thing to absorb the rank-entry barrier
  - program order puts iter-0 s-matmuls before all setup so AR0 starts ASAP
"""

import os
import numpy as np

B, R, C, D, I = 256, 1152, 10, 16, 8
CD = C * D                 # 160
NCORES = 8
RL = R // NCORES           # 144 routes per core
NCHUNK = RL * I // 128     # 9 local K-chunks of 128
GCHUNK = R * I // 128      # 72 global K-chunks (for the replicated iter-0)
NPIECE = 4                 # input blob split for DMA/compute overlap
GPP = GCHUNK // NPIECE     # 18 global chunks per piece
KMAIN = 8                  # chunks whose routes live in b_main (8*16=128)
NITER = 3

_CACHE = {}


def _build_program():
    from contextlib import ExitStack

    import concourse.bass as bass
    import concourse.bacc as bacc
    import concourse.mybir as mybir
    import concourse.tile as tile

    f32 = mybir.dt.float32
    bf16 = mybir.dt.bfloat16
    AF = mybir.ActivationFunctionType
    ALU = mybir.AluOpType

    nc = bacc.Bacc(None, num_devices=NCORES)

    # bf16 pieces: per piece GPP chunks of uT + matching W, own chunks first
    PW = GPP * (B + CD)       # 7488 cols per piece
    # f32 blob: local Wt | mask2  (cols 1440 | 16)
    o_Wt, o_mk = 0, NCHUNK * CD
    DWF = o_mk + 16
    data_p = [
        nc.declare_dram_parameter(f"data_p{p}", [128, PW], bf16, isOutput=False)
        for p in range(NPIECE)
    ]
    data_f = nc.declare_dram_parameter("data_f", [128, DWF], f32, isOutput=False)
    data_s = nc.declare_dram_parameter("data_s", [128, NCHUNK * 128], bf16,
                                       isOutput=False)
    out_d = nc.declare_dram_parameter("out", [B, CD], f32, isOutput=True)

    rgroups = [list(range(NCORES))]

    with tile.TileContext(nc) as tc, ExitStack() as ctx:
        singles = ctx.enter_context(tc.tile_pool(name="singles", bufs=1))
        wcpool = ctx.enter_context(tc.tile_pool(name="wc", bufs=3))
        stpool = ctx.enter_context(tc.tile_pool(name="stage", bufs=2))
        work = ctx.enter_context(tc.tile_pool(name="work", bufs=2))
        ps_s = ctx.enter_context(tc.tile_pool(name="ps_s", bufs=1, space="PSUM"))
        ps_e = ctx.enter_context(tc.tile_pool(name="ps_e", bufs=2, space="PSUM"))
        ps_z = ctx.enter_context(tc.tile_pool(name="ps_z", bufs=1, space="PSUM"))
        ps_b = ctx.enter_context(tc.tile_pool(name="ps_b", bufs=1, space="PSUM"))
        ps_t = ctx.enter_context(tc.tile_pool(name="ps_t", bufs=1, space="PSUM"))
        dram = ctx.enter_context(tc.tile_pool(name="dram", bufs=1, space="DRAM"))

        # --- input DMAs, piecewise so s0 matmuls overlap later pieces
        sb_p = []
        for p in range(NPIECE):
            t = singles.tile([128, PW], bf16, tag=f"data_p{p}")
            nc.sync.dma_start(out=t, in_=data_p[p][:])
            sb_p.append(t)
        sb_f = singles.tile([128, DWF], f32, tag="data_f")
        nc.sync.dma_start(out=sb_f, in_=data_f[:])
        sb_sel = singles.tile([128, NCHUNK * 128], bf16, tag="data_s")
        nc.sync.dma_start(out=sb_sel, in_=data_s[:])
        # local views (own chunks are the first NCHUNK of piece 0)
        sb_uTb = sb_p[0][:, 0:GPP * B]
        sb_Wt = sb_f[:, o_Wt:o_Wt + NCHUNK * CD]
        sb_mask = sb_f[:, o_mk:o_mk + 16]

        # collective buffers for iterations 1, 2
        cc = [None]
        for it in (1, 2):
            w = 3 * CD
            ci = dram.tile([128, w], bf16, tag=f"cc_in{it}", name=f"cc_in{it}")
            co = dram.tile([128, w], bf16, tag=f"cc_out{it}", name=f"cc_out{it}")
            cc.append((ci, co, w))

        # --- replicated iteration 0: full-R s0 on every core, no collective
        # (separate PSUM tiles per bh: start=True clears has_written for the
        # whole BANK, so interleaved accumulation chains must not share one)
        st = [ps_s.tile([128, CD], f32, tag=f"s{bh}", name=f"s0_{bh}")
              for bh in range(2)]
        for p in range(NPIECE):
            for k in range(GPP):
                for bh in range(2):
                    nc.tensor.matmul(
                        st[bh],
                        sb_p[p][:, k * B + bh * 128: k * B + (bh + 1) * 128],
                        sb_p[p][:, GPP * B + k * CD: GPP * B + (k + 1) * CD],
                        start=(p == 0 and k == 0),
                        stop=(p == NPIECE - 1 and k == GPP - 1),
                    )
        red0 = stpool.tile([128, 2 * CD], bf16, tag="red", name="red0")
        for bh in range(2):
            nc.vector.tensor_copy(
                out=red0[:, bh * CD:(bh + 1) * CD], in_=st[bh]
            )

        # --- setup, scheduled after AR0's trigger -> hidden under the AR
        sb_ones = singles.tile([128, 128], bf16, tag="ones")
        nc.vector.memset(sb_ones, 1.0)

        # Wd[(rp,i), (k,c)] = (1/B) * sum_d Wt
        sb_Wd = singles.tile([128, NCHUNK * C], f32, tag="Wd")
        for k in range(NCHUNK):
            nc.vector.reduce_sum(
                out=sb_Wd[:, k * C:(k + 1) * C],
                in_=sb_Wt[:, k * CD:(k + 1) * CD].rearrange("p (c d) -> p c d", d=D),
                axis=mybir.AxisListType.X,
            )
        nc.vector.tensor_scalar_mul(sb_Wd, sb_Wd, 1.0 / B)

        # Wdbd[p, (k, c, rp)] = Wd[p, (k,c)] * mask2[p, rp]   (block-diagonal)
        sb_Wdbd = singles.tile([128, NCHUNK * CD], bf16, tag="Wdbd")
        wd_b = bass.AP(
            tensor=sb_Wd.tensor, offset=sb_Wd.offset,
            ap=[sb_Wd.ap[0], [C, NCHUNK], [1, C], [0, 16]],
        )
        mk_b = bass.AP(
            tensor=sb_mask.tensor, offset=sb_mask.offset,
            ap=[sb_mask.ap[0], [0, NCHUNK], [0, C], [1, 16]],
        )
        nc.vector.tensor_mul(
            sb_Wdbd.rearrange("p (k c rp) -> p k c rp", c=C, rp=16), wd_b, mk_b
        )

        # t[b, (c, k, rp)] = sum_i Wd[(rp,i),(k,c)] u[b, r(k,rp), i]
        sb_t = [singles.tile([128, C * RL], bf16, tag=f"t{bh}", name=f"t{bh}")
                for bh in range(2)]
        t_r = [sb_t[bh].rearrange("p (c k rp) -> p k c rp", k=NCHUNK, rp=16)
               for bh in range(2)]
        for k in range(NCHUNK):
            for bh in range(2):
                pt = ps_t.tile([128, CD], f32, tag="pt", name="pt")
                nc.tensor.matmul(
                    pt,
                    sb_uTb[:, k * B + bh * 128: k * B + (bh + 1) * 128],
                    sb_Wdbd[:, k * CD:(k + 1) * CD],
                    start=True, stop=True,
                )
                nc.vector.tensor_copy(
                    out=t_r[bh][:, k],
                    in_=pt.rearrange("p (c rp) -> p c rp", rp=16),
                )

        # b_ij: main [128=(k<8,rp), (c,d)] f32 + tail [16=rp(k=8), (c,d)] f32
        b_main = singles.tile([128, CD], f32, tag="bm")
        nc.vector.memset(b_main, 0.0)
        b_tail = singles.tile([16, CD], f32, tag="bt")
        nc.vector.memset(b_tail, 0.0)
        E_main = singles.tile([128, CD], bf16, tag="Em")
        E_tail = singles.tile([16, CD], bf16, tag="Et")
        sb_vb = singles.tile([128, 2 * CD], bf16, tag="vb")
        sb_vf = singles.tile([128, 2 * CD], f32, tag="vf")

        for it in range(NITER):
            # ---- v = y|y| / (den) from the AllReduce result of iteration it
            red = red0 if it == 0 else red
            y = red[:, 0:2 * CD]
            sq = work.tile([128, 2 * CD], f32, tag="sq")
            nc.vector.tensor_mul(sq, y, y)
            negy = work.tile([128, 2 * CD], bf16, tag="negy")
            nc.vector.tensor_scalar_mul(negy, y, -1.0)
            absy = work.tile([128, 2 * CD], bf16, tag="absy")
            nc.vector.tensor_tensor(out=absy, in0=y, in1=negy, op=ALU.max)
            den = work.tile([128, 2 * CD], f32, tag="den")
            if it == 0:
                nc.vector.tensor_scalar_add(den, sq, float(R) * float(R))
            else:
                zr = red[:, 2 * CD:3 * CD]
                z2 = work.tile([128, CD], f32, tag="z2")
                nc.vector.tensor_mul(z2, zr, zr)
                z2b = bass.AP(
                    tensor=z2.tensor, offset=z2.offset,
                    ap=[z2.ap[0], [0, 2], [1, CD]],
                )
                nc.vector.tensor_add(
                    den.rearrange("p (h f) -> p h f", f=CD),
                    sq.rearrange("p (h f) -> p h f", f=CD),
                    z2b,
                )
            rec = work.tile([128, 2 * CD], f32, tag="rec")
            nc.vector.reciprocal_approx_fast(out=rec, in_=den)
            num = work.tile([128, 2 * CD], f32, tag="num")
            nc.vector.tensor_mul(num, y, absy)
            if it < NITER - 1:
                nc.vector.tensor_mul(sb_vb, num, rec)
            else:
                nc.vector.tensor_mul(sb_vf, num, rec)
                for bh in range(2):
                    nc.sync.dma_start(
                        out=out_d[bh * 128:(bh + 1) * 128, :],
                        in_=sb_vf[:, bh * CD:(bh + 1) * CD],
                    )
                break

            # ---- b-update: 20 main (M=128) + 20 tail (M=16) matmuls
            pb = ps_b.tile([128, CD], f32, tag="pb", name="pb")
            pbt = ps_b.tile([16, CD], f32, tag="pbt", name="pbt")
            for c in range(C):
                for bh in range(2):
                    nc.tensor.matmul(
                        pb[:, c * D:(c + 1) * D],
                        sb_t[bh][:, c * RL: c * RL + 128],
                        sb_vb[:, bh * CD + c * D: bh * CD + (c + 1) * D],
                        start=(bh == 0), stop=(bh == 1),
                    )
            for c in range(C):
                for bh in range(2):
                    nc.tensor.matmul(
                        pbt[:, c * D:(c + 1) * D],
                        sb_t[bh][:, c * RL + 128: (c + 1) * RL],
                        sb_vb[:, bh * CD + c * D: bh * CD + (c + 1) * D],
                        start=(bh == 0), stop=(bh == 1),
                    )
            nc.vector.tensor_add(b_main, b_main, pb)
            nc.vector.tensor_add(b_tail, b_tail, pbt)

            # ---- next iteration: E = exp(b), Z-partial, Weff, s-matmuls
            nc.scalar.activation(out=E_main, in_=b_main, func=AF.Exp)
            nc.scalar.activation(out=E_tail, in_=b_tail, func=AF.Exp)
            pz = ps_z.tile([128, CD], f32, tag="pz", name="pz")
            nc.tensor.matmul(pz, sb_ones, E_main, start=True, stop=False)
            nc.tensor.matmul(pz, sb_ones[0:16, :], E_tail, start=False, stop=True)

            ci, co, w = cc[it + 1]
            stage = stpool.tile([128, w], bf16, tag="stage", name=f"stage{it+1}")
            nc.scalar.activation(
                out=stage[:, 2 * CD:3 * CD], in_=pz, func=AF.Copy
            )
            st = [ps_s.tile([128, CD], f32, tag=f"s{bh}", name=f"s{it+1}_{bh}")
                  for bh in range(2)]

            def emit_pe(g):
                pe = ps_e.tile([128, 3 * CD], f32, tag="pe", name="pe")
                for j in range(3):
                    k = 3 * g + j
                    if k < KMAIN:
                        nc.tensor.matmul(
                            pe[:, j * CD:(j + 1) * CD],
                            sb_sel[:, k * 128:(k + 1) * 128], E_main,
                            start=True, stop=True,
                        )
                    else:
                        nc.tensor.matmul(
                            pe[:, j * CD:(j + 1) * CD],
                            sb_sel[0:16, KMAIN * 128:(KMAIN + 1) * 128], E_tail,
                            start=True, stop=True,
                        )
                return pe

            def emit_st(g, pe):
                rhs = wcpool.tile([128, 3 * CD], bf16, tag="wc")
                nc.vector.tensor_mul(
                    rhs, sb_Wt[:, 3 * g * CD:(3 * g + 3) * CD], pe
                )
                for j in range(3):
                    k = 3 * g + j
                    for bh in range(2):
                        nc.tensor.matmul(
                            st[bh],
                            sb_uTb[:, k * B + bh * 128: k * B + (bh + 1) * 128],
                            rhs[:, j * CD:(j + 1) * CD],
                            start=(k == 0), stop=(k == NCHUNK - 1),
                        )

            pe0 = emit_pe(0)
            pe1 = emit_pe(1)
            emit_st(0, pe0)
            pe2 = emit_pe(2)
            emit_st(1, pe1)
            emit_st(2, pe2)

            for bh in range(2):
                nc.vector.tensor_copy(
                    out=stage[:, bh * CD:(bh + 1) * CD], in_=st[bh]
                )
            nc.sync.dma_start(out=ci[:], in_=stage)
            nc.gpsimd.collective_compute(
                "AllReduce", ALU.add,
                replica_groups=rgroups, ins=[ci.opt()], outs=[co.opt()],
            )
            red = stpool.tile([128, w], bf16, tag="red", name=f"red{it+1}")
            nc.sync.dma_start(out=red, in_=co[:])

    nc.compile()
    return nc


def _host_inputs(u, W):
    """Pure permutation + dtype cast host prep: per-core (r,i)-major layouts."""
    import concourse.mybir as mybir

    bf16 = mybir.dt.np(mybir.dt.bfloat16)
    u = np.ascontiguousarray(u, dtype=np.float32)
    W = np.ascontiguousarray(W, dtype=np.float32)

    o_Wt, o_mk = 0, NCHUNK * CD
    DWF = o_mk + 16

    sel = np.zeros((128, NCHUNK * 128), dtype=np.float32)
    for k in range(KMAIN):
        for rp in range(16):
            sel[k * 16 + rp, k * 128 + rp * 8:(k * 128 + rp * 8) + 8] = 1.0
    for rp in range(16):
        sel[rp, KMAIN * 128 + rp * 8: KMAIN * 128 + rp * 8 + 8] = 1.0
    mask2 = np.zeros((128, 16), dtype=np.float32)
    for p in range(128):
        mask2[p, p // 8] = 1.0

    # global (r,i)-major chunk tensors, shared across cores
    uT_g = u.reshape(B, R * I).T.reshape(GCHUNK, 128, B)        # (72,128,256)
    Wt_g = (W.transpose(0, 3, 1, 2).reshape(R * I, CD)
            .reshape(GCHUNK, 128, CD))                          # (72,128,160)
    PW = GPP * (B + CD)

    in_maps = []
    for ci in range(NCORES):
        order = np.roll(np.arange(GCHUNK), -NCHUNK * ci)        # own chunks first
        uT_r = uT_g[order]
        Wt_r = Wt_g[order]
        m = {}
        for p in range(NPIECE):
            blob = np.zeros((128, PW), dtype=np.float32)
            sl = slice(p * GPP, (p + 1) * GPP)
            blob[:, 0:GPP * B] = (
                uT_r[sl].transpose(1, 0, 2).reshape(128, GPP * B)
            )
            blob[:, GPP * B:] = (
                Wt_r[sl].transpose(1, 0, 2).reshape(128, GPP * CD)
            )
            m[f"data_p{p}"] = blob.astype(bf16)
        df = np.zeros((128, DWF), dtype=np.float32)
        df[:, o_Wt:o_Wt + NCHUNK * CD] = (
            Wt_r[0:NCHUNK].transpose(1, 0, 2).reshape(128, NCHUNK * CD)
        )
        df[:, o_mk:o_mk + 16] = mask2
        m["data_f"] = df
        m["data_s"] = sel.astype(bf16)
        in_maps.append(m)
    return in_maps


def _install_profile_hook():
    """Recreate the missing antenv.axon_hooks NTFF-profile hook (dev only)."""
    import contextlib
    import ctypes
    import sys
    import types

    try:
        from antenv.axon_hooks import get_axon_ntff_profile_hook  # noqa: F401
        return
    except ImportError:
        pass

    mod = types.ModuleType("antenv.axon_hooks")
    holder = {}
    mod.set_axon_ntff_profile_hook = lambda h: holder.__setitem__("h", h)
    mod.get_axon_ntff_profile_hook = lambda: holder.get("h")
    import antenv

    sys.modules["antenv.axon_hooks"] = mod
    antenv.axon_hooks = mod

    so_path = "/opt/axon/libaxon_pjrt.so"
    lib = ctypes.CDLL(so_path)
    if not hasattr(lib, "axon_start_nrt_profile"):
        return
    lib.axon_start_nrt_profile.argtypes = [
        ctypes.POINTER(ctypes.c_int64),
        ctypes.c_size_t,
    ]
    lib.axon_start_nrt_profile.restype = ctypes.c_int64
    lib.axon_stop_nrt_profile.argtypes = [ctypes.c_char_p]
    lib.axon_stop_nrt_profile.restype = ctypes.c_int64

    @contextlib.contextmanager
    def _hook(output_dir, device_ids):
        import jax

        jax.devices()
        if device_ids:
            ids = (ctypes.c_int64 * len(device_ids))(*device_ids)
            rc = lib.axon_start_nrt_profile(ids, len(device_ids))
        else:
            rc = lib.axon_start_nrt_profile(None, 0)
        if rc != 0:
            raise RuntimeError(f"axon_start_nrt_profile rc={rc}")
        try:
            yield
        finally:
            n = lib.axon_stop_nrt_profile(str(output_dir).encode())
            print(f"profile: {n} file(s) written to {output_dir}")

    mod.set_axon_ntff_profile_hook(_hook)

    import concourse.bass_utils as bu

    bu.upload_artifacts = lambda tmpdir: f"local:{tmpdir}"


def kernel(u, W):
    from concourse.bass_utils import run_bass_kernel_spmd

    if os.environ.get("KERNEL_TRACE", "0") == "1":
        _install_profile_hook()
    if "nc" not in _CACHE:
        _CACHE["nc"] = _build_program()
    nc = _CACHE["nc"]
    in_maps = _host_inputs(u, W)
    trace = os.environ.get("KERNEL_TRACE", "0") == "1"
    res = run_bass_kernel_spmd(
        nc, in_maps, core_ids=list(range(NCORES)), trace=trace
    )
    _CACHE["last_result"] = res
    return np.asarray(res.results[0]["out"]).reshape(B, C, D)
